# revision 1
# baseline (speedup 1.0000x reference)
"""ContextQueryAttention (BiDAF trilinear attention) on 8 Trainium2 NeuronCores.

Full inputs:  c (32, 2048, 128) f32, q (32, 256, 128) f32, W0 (384,) f32
Full output:  (32, 2048, 512) f32 = concat([c, A, c*A, c*Bm], -1)

Sharding: pure data parallel — batch 32 is split 4-per-core across 8 cores;
every contraction is per-batch so there is no cross-core communication.

Per-batch math, with w1,w2,w3 = W0 split in 3, G = (c*w3) @ q^T:
    S[i,j] = c_i.w1 + q_j.w2 + G[i,j]
    S1 = softmax_j(S); S2 = softmax_i(S)
    A = S1 @ q ; T = S2^T @ c ; Bm = S1 @ T
Softmax normalization is linear in the contractions, so S is never
normalized on-chip.  With cw1[i] = c_i.w1 and g[j] = exp(q_j.w2):
    F_nat = exp(G + cw1)   natural [i,j] layout  (g factor cancels in T)
    F_t   = exp(G^T)       [j,i] layout          (exp(cw1) cancels in A/Bm)
    [UT|s'][j] = sum_i F_nat[i,j] * [c_i | 1]          -> T = UT/s'
    [UA|UBm|r][i] = sum_j F_t[j,i] * [g*q | g*T | g]   -> A = UA/r, Bm = UBm/r
"""

from contextlib import ExitStack

import numpy as np

import concourse.bacc as bacc
import concourse.mybir as mybir
import concourse.tile as tile
from concourse.bass_utils import run_bass_kernel_spmd

F32 = mybir.dt.float32
BF16 = mybir.dt.bfloat16
P = 128

N_CORES = 8
B, LC, LQ, D = 32, 2048, 256, 128
BC = B // N_CORES


def _build_nc(BC=BC, LC=LC, LQ=LQ, D=D):
    NT = LC // P          # i-chunks
    NJ = LQ // P          # j-halves
    NBLK = LC // 512      # i-blocks for the transposed S matmul

    nc = bacc.Bacc("TRN2", target_bir_lowering=False, debug=False)
    c_d = nc.dram_tensor("c", [BC, LC, D], F32, kind="ExternalInput").ap()
    q_d = nc.dram_tensor("q", [BC, LQ, D], F32, kind="ExternalInput").ap()
    w_d = nc.dram_tensor("W0", [3 * D], F32, kind="ExternalInput").ap()
    o_d = nc.dram_tensor("out", [BC, LC, 4 * D], F32, kind="ExternalOutput").ap()

    with tile.TileContext(nc) as tc, ExitStack() as ctx:
        singles = ctx.enter_context(tc.tile_pool(name="singles", bufs=1))
        cpool = ctx.enter_context(tc.tile_pool(name="cpool", bufs=2))
        qpool = ctx.enter_context(tc.tile_pool(name="qpool", bufs=2))
        fpool = ctx.enter_context(tc.tile_pool(name="fpool", bufs=3))
        ftpool = ctx.enter_context(tc.tile_pool(name="ftpool", bufs=4))
        opool = ctx.enter_context(tc.tile_pool(name="opool", bufs=3))
        small = ctx.enter_context(tc.tile_pool(name="small", bufs=4))
        psG = ctx.enter_context(tc.tile_pool(name="psG", bufs=2, space="PSUM"))
        psGT = ctx.enter_context(tc.tile_pool(name="psGT", bufs=2, space="PSUM"))
        psUT = ctx.enter_context(tc.tile_pool(name="psUT", bufs=2, space="PSUM"))
        psAB = ctx.enter_context(tc.tile_pool(name="psAB", bufs=2, space="PSUM"))

        # ---- one-time: weight vectors as per-partition columns [128, 3] ----
        w_sb = singles.tile([P, 3], F32)
        nc.gpsimd.dma_start(out=w_sb, in_=w_d.rearrange("(k p) -> p k", p=P))
        w_bf = singles.tile([P, 3], BF16)
        nc.vector.tensor_copy(out=w_bf, in_=w_sb)
        w1bf = w_bf[:, 0:1]
        w2bf = w_bf[:, 1:2]

        for b in range(BC):
            # ---------------- loads / layout prep ----------------
            cn32 = cpool.tile([P, NT, D], F32, tag="cn32")
            nc.sync.dma_start(out=cn32, in_=c_d[b].rearrange("(t p) d -> p t d", p=P))
            # bf16 copy of c with a ones column appended ([c | 1], 129 wide)
            cne = cpool.tile([P, NT, D + 1], BF16, tag="cne")
            nc.vector.tensor_copy(out=cne[:, :, 0:D], in_=cn32)
            nc.vector.memset(cne[:, :, D : D + 1], 1.0)
            # c^T (bf16) via DMA transpose: ct16[:, t, :] = c[t-chunk].T
            ct16 = cpool.tile([P, NT, P], BF16, tag="ct16")
            for t in range(NT):
                nc.sync.dma_start_transpose(out=ct16[:, t, :], in_=cne[:, t, 0:D])

            qn32 = qpool.tile([P, NJ, D], F32, tag="qn32")
            nc.sync.dma_start(out=qn32, in_=q_d[b].rearrange("(h p) d -> p h d", p=P))
            qn16 = qpool.tile([P, NJ, D], BF16, tag="qn16")
            nc.vector.tensor_copy(out=qn16, in_=qn32)
            qt16 = qpool.tile([P, NJ, P], BF16, tag="qt16")
            for h in range(NJ):
                nc.sync.dma_start_transpose(out=qt16[:, h, :], in_=qn16[:, h, :])

            # rhsG = [w3*q^T | w1]  (d on partitions, 257 wide)
            rhsG = qpool.tile([P, LQ + 1], BF16, tag="rhsG")
            nc.vector.tensor_scalar_mul(
                out=rhsG[:, 0:LQ],
                in0=qt16.rearrange("p h j -> p (h j)"),
                scalar1=w_sb[:, 2:3],
            )
            nc.vector.tensor_copy(out=rhsG[:, LQ : LQ + 1], in_=w1bf)

            # g = exp(q.w2) per j  (two [128,1] halves)
            qw2_ps = psG.tile([P, NJ], F32, tag="gps")
            for h in range(NJ):
                nc.tensor.matmul(
                    out=qw2_ps[:, h : h + 1], lhsT=qt16[:, h, :], rhs=w2bf
                )
            g_sb = small.tile([P, NJ], F32, tag="g")
            nc.scalar.activation(
                out=g_sb, in_=qw2_ps, func=mybir.ActivationFunctionType.Exp
            )
            g_bf = small.tile([P, NJ], BF16, tag="gbf")
            nc.vector.tensor_copy(out=g_bf, in_=g_sb)

            # rhs for the A/Bm matmul: [g*q | g*T | g], filled per half.
            qs16 = qpool.tile([P, NJ, 2 * D + 1], BF16, tag="qs16")
            for h in range(NJ):
                nc.vector.tensor_scalar_mul(
                    out=qs16[:, h, 0:D], in0=qn16[:, h, :], scalar1=g_sb[:, h : h + 1]
                )
                nc.vector.tensor_copy(
                    out=qs16[:, h, 2 * D : 2 * D + 1], in_=g_bf[:, h : h + 1]
                )

            # ------------- natural side: F_nat + UT accumulation -------------
            ut_ps = [
                psUT.tile([P, D + 1], F32, tag="ut", name=f"ut{h}") for h in range(NJ)
            ]
            for t in range(NT):
                gps = psG.tile([P, LQ + 1], F32, tag="gps")
                nc.tensor.matmul(out=gps, lhsT=ct16[:, t, :], rhs=rhsG)
                cw1_sb = small.tile([P, 1], F32, tag="cw1")
                nc.vector.tensor_copy(out=cw1_sb, in_=gps[:, LQ : LQ + 1])
                fn = fpool.tile([P, LQ], BF16, tag="fn")
                nc.scalar.activation(
                    out=fn,
                    in_=gps[:, 0:LQ],
                    func=mybir.ActivationFunctionType.Exp,
                    bias=cw1_sb,
                )
                for h in range(NJ):
                    nc.tensor.matmul(
                        out=ut_ps[h],
                        lhsT=fn[:, h * P : (h + 1) * P],
                        rhs=cne[:, t, :],
                        start=(t == 0),
                        stop=(t == NT - 1),
                    )

            # ------------- transposed side: F_t = exp(G^T) -------------
            ft = [
                ftpool.tile([P, LC], BF16, tag="ft", name=f"ft{h}") for h in range(NJ)
            ]
            for h in range(NJ):
                for blk in range(NBLK):
                    gt_ps = psGT.tile([P, 512], F32, tag="gt")
                    nc.tensor.matmul(
                        out=gt_ps,
                        lhsT=rhsG[:, h * P : (h + 1) * P],
                        rhs=ct16.rearrange("p t i -> p (t i)")[
                            :, blk * 512 : (blk + 1) * 512
                        ],
                    )
                    nc.scalar.activation(
                        out=ft[h][:, blk * 512 : (blk + 1) * 512],
                        in_=gt_ps,
                        func=mybir.ActivationFunctionType.Exp,
                    )

            # ------------- T~ = g * UT / s'  (into qs16 cols 128:256) -------------
            for h in range(NJ):
                recip_s = small.tile([P, 1], F32, tag="rs")
                nc.vector.reciprocal(out=recip_s, in_=ut_ps[h][:, D : D + 1])
                scale_j = small.tile([P, 1], F32, tag="sj")
                nc.vector.tensor_mul(out=scale_j, in0=g_sb[:, h : h + 1], in1=recip_s)
                nc.vector.tensor_scalar_mul(
                    out=qs16[:, h, D : 2 * D], in0=ut_ps[h][:, 0:D], scalar1=scale_j
                )

            # ------------- A/Bm + epilogue per i-chunk -------------
            for t in range(NT):
                ab_ps = psAB.tile([P, 2 * D + 1], F32, tag="ab")
                for h in range(NJ):
                    nc.tensor.matmul(
                        out=ab_ps,
                        lhsT=ft[h][:, t * P : (t + 1) * P],
                        rhs=qs16[:, h, :],
                        start=(h == 0),
                        stop=(h == NJ - 1),
                    )
                recip_r = small.tile([P, 1], F32, tag="rr")
                nc.vector.reciprocal(out=recip_r, in_=ab_ps[:, 2 * D : 2 * D + 1])
                out_sb = opool.tile([P, 3 * D], F32, tag="osb")
                # A
                nc.vector.tensor_scalar_mul(
                    out=out_sb[:, 0:D], in0=ab_ps[:, 0:D], scalar1=recip_r
                )
                # c*A
                nc.vector.tensor_mul(
                    out=out_sb[:, D : 2 * D], in0=out_sb[:, 0:D], in1=cn32[:, t, :]
                )
                # Bm, then c*Bm
                bm_sb = small.tile([P, D], F32, tag="bm")
                nc.vector.tensor_scalar_mul(
                    out=bm_sb, in0=ab_ps[:, D : 2 * D], scalar1=recip_r
                )
                nc.vector.tensor_mul(
                    out=out_sb[:, 2 * D : 3 * D], in0=bm_sb, in1=cn32[:, t, :]
                )
                # out = [c | A | c*A | c*Bm]
                nc.sync.dma_start(
                    out=o_d[b, t * P : (t + 1) * P, 0:D], in_=cn32[:, t, :]
                )
                nc.sync.dma_start(
                    out=o_d[b, t * P : (t + 1) * P, D : 4 * D], in_=out_sb
                )

    nc.finalize()
    return nc


_NC_CACHE = None


def _get_nc():
    global _NC_CACHE
    if _NC_CACHE is None:
        _NC_CACHE = _build_nc()
    return _NC_CACHE


def run(c, q, W0, trace=False):
    c = np.ascontiguousarray(np.asarray(c, dtype=np.float32))
    q = np.ascontiguousarray(np.asarray(q, dtype=np.float32))
    W0 = np.ascontiguousarray(np.asarray(W0, dtype=np.float32))
    assert c.shape == (B, LC, D) and q.shape == (B, LQ, D) and W0.shape == (3 * D,)

    nc = _get_nc()
    in_maps = [
        {"c": c[k * BC : (k + 1) * BC], "q": q[k * BC : (k + 1) * BC], "W0": W0}
        for k in range(N_CORES)
    ]
    res = run_bass_kernel_spmd(nc, in_maps, core_ids=list(range(N_CORES)), trace=trace)
    out = np.concatenate([res.results[k]["out"] for k in range(N_CORES)], axis=0)
    return out, res


def kernel(c, q, W0):
    out, _ = run(c, q, W0, trace=False)
    return out
